# revision 5
# baseline (speedup 1.0000x reference)
"""Pairwise squared-Euclidean distance map on 8 TRN2 NeuronCores.

d[b, i, j] = sum_c (a[b, c, i] - b[b, c, j])^2
           = aa[b, i] + bb[b, j] - 2 * <a[b, :, i], b[b, :, j]>

Sharding: data-parallel over the N dimension (rows of the distance map).
Core k computes d[:, k*512:(k+1)*512, :] from a[:, :, k*512:(k+1)*512]
and the full (small) b tensor.

All prep happens ON THE HOST: numpy computes aa/bb and assembles fp8
(e4m3) augmented operands with hi/lo splitting so the fp8 quantization
error cancels to second order:
    cross = (-2a)b = c_hi.b_hi + c_hi.b_lo + c_lo.b_hi (+ dropped 2nd-order)
plus hi/lo norm rows and a ones*128 row: K = 3*64 + 7 + pad = 200
contraction rows, folded [100, 2, *] for the TensorE DoubleRow perf
mode (KI=100 spans all four 32-row PE groups, required for full rate).

Trace-informed structure (ntff profiles of the 73-80us baselines):
- ~7.2 us fixed Tile/runtime preamble before any engine can issue a DMA.
- The whole chip (PE clock AND DMA/fabric clock) ramps with sustained
  activity: a HAM grant ~12 us after gapless execution begins unlocks
  full speed; any engine stall pushes the grant out and holds DMA at
  ~50%. So the schedule is built to be GAPLESS from the first matmul.
- Critical loads (lhs0 + batch-0 rhs chunks) ride the sync HWDGE FIFO
  alone, in need-order; batch 0 runs CHUNK-MAJOR across its 4 row
  blocks so chunk k isn't needed until ~8 matmul-pairs after chunk
  k-1, tolerating the slow pre-grant DMA ramp without PE gaps.
- Batches 1-3 load on gpsimd SWDGE, gated behind block-0's first drain
  by true WAW deps (a 1-element pre-write into each DMA's destination,
  chained off the stage tile): the Tile scheduler reorders per-engine
  program order, so only real data deps can hold SWDGE back (measured:
  ungated SWDGE steals 2-4x the round-robin share and starves the
  gating loads, stalling the PE ~7 us and delaying the HAM grant).
- 128 DoubleRow matmuls (512 cols each, 2 per 2-bank PSUM tile).
- Drains are the mid-kernel pacer: PSUM->SBUF fp16 at 1 elem/cycle.
  1024-wide drains measure 1223 ns on Vector and 1114 ns on Scalar;
  greedy cumulative balance instead of strict alternation (alternation
  idles the faster engine).
- fp16 stores on the sync HWDGE queue behind the loads: 1024-wide for
  batch 0 (start the stream early) and the last block (short final
  receipt), 4096-wide elsewhere. The whole 16 MB output is staged in
  SBUF so stores never backpressure the PE. DMA sustains ~420 GB/s
  post-grant; total 20.5 MB of HBM traffic is the floor.
"""

import numpy as np
from contextlib import ExitStack

import concourse.bass as bass
import concourse.bacc as bacc
import concourse.mybir as mybir
from concourse.tile import TileContext
from concourse.bass_utils import run_bass_kernel_spmd

B, C, N, M = 4, 64, 4096, 4096
NCORES = 8
NSH = N // NCORES          # 512 N rows per core
NB = NSH // 128            # 4 row blocks of 128
MC = 512                   # output cols per DoubleRow matmul (1 PSUM bank)
DW = 1024                  # drain width (2 PSUM banks per drain)
KAUG = 200                 # padded contraction rows
KI = KAUG // 2             # folded partition rows for DoubleRow
MCH = 1024                 # rhs chunk width (cols)
NCH = M // MCH             # 4 chunks per batch

F32 = mybir.dt.float32
F16 = mybir.dt.float16
F8 = mybir.dt.float8e4

_CACHE = {}


def _build_nc():
    nc = bacc.Bacc(
        "TRN2",
        target_bir_lowering=False,
        debug=False,
        enable_asserts=True,
        num_devices=NCORES,
    )
    lhs_d = nc.declare_dram_parameter("lhs", [KI, B, 2, NSH], F8, isOutput=False)
    rhs_d = nc.declare_dram_parameter(
        "rhs", [KI, B, NCH, 2, MCH], F8, isOutput=False
    )
    d_d = nc.declare_dram_parameter("d", [B, NSH, M], F16, isOutput=True)

    DR = mybir.MatmulPerfMode.DoubleRow

    with ExitStack() as ctx:
        tc = ctx.enter_context(TileContext(nc))
        lpool = ctx.enter_context(tc.tile_pool(name="lhs", bufs=1))
        rpool = ctx.enter_context(tc.tile_pool(name="rhs", bufs=1))
        gpool = ctx.enter_context(tc.tile_pool(name="gate", bufs=1))
        stage = ctx.enter_context(tc.tile_pool(name="stage", bufs=16))
        mpsum = ctx.enter_context(tc.tile_pool(name="mpsum", bufs=4, space="PSUM"))

        lts = lpool.tile([KI, B, 2, NSH], F8, tag="lt", name="lt")
        rtc = rpool.tile([KI, B, NCH, 2, MCH], F8, tag="rt", name="rt")
        g16 = gpool.tile([1, 2], F16, tag="gt", name="gt")

        # Critical loads in need-order on the sync HWDGE FIFO; nothing
        # else rides this queue until the stores, so the mm0 gates get
        # the full (ramping) DMA bandwidth the moment the preamble ends.
        nc.sync.dma_start(out=lts[:, 0], in_=lhs_d[:, 0])
        nc.sync.dma_start(out=rtc[:, 0, 0, :, 0:MC], in_=rhs_d[:, 0, 0, :, 0:MC])
        nc.sync.dma_start(
            out=rtc[:, 0, 0, :, MC:MCH], in_=rhs_d[:, 0, 0, :, MC:MCH]
        )
        for ch in range(1, NCH):
            nc.sync.dma_start(out=rtc[:, 0, ch], in_=rhs_d[:, 0, ch])

        # Deferred loads (batches 1-3): issued on gpsimd SWDGE, each held
        # behind the ARRIVAL of batch-0 chunk 1 by a true dep chain: g16
        # reads a corner of the c1 region (RAW on that DMA), then a
        # 1-element pre-write into each deferred DMA's destination
        # region (RAW on g16, then WAW with the DMA). The DMA data
        # overwrites the pre-written bytes. Granularity is per-chunk in
        # need order so batch 1's first chunk lands ~8 us before its
        # first matmul even at the shared early DMA rate.
        def issue_deferred():
            nc.gpsimd.tensor_copy(
                g16[0:1, 0:1], rtc[0:1, 0, 1, 0, 0:2].bitcast(F16)
            )
            targets = [
                (lts[0:1, 1, 0, 0:2], lts[:, 1:B], lhs_d[:, 1:B]),
            ] + [
                (rtc[0:1, 1, ch, 0, 0:2], rtc[:, 1, ch], rhs_d[:, 1, ch])
                for ch in range(NCH)
            ] + [
                (rtc[0:1, bt, 0, 0, 0:2], rtc[:, bt], rhs_d[:, bt])
                for bt in range(2, B)
            ]
            for corner, dst, src in targets:
                nc.gpsimd.tensor_copy(corner.bitcast(F16), g16[0:1, 0:1])
                nc.gpsimd.dma_start(out=dst, in_=src)

        issue_deferred()

        # Greedy drain balance with measured per-1024-col drain costs:
        # 1223 ns on Vector, 1114 ns on Scalar.
        bal = [0.0, 0.0]

        def drain(dst, src):
            if bal[0] + 1223 <= bal[1] + 1114:
                bal[0] += 1223
                nc.vector.tensor_copy(dst, src)
            else:
                bal[1] += 1114
                nc.scalar.copy(dst, src)

        def mm_pair(pt, bt, i, q):
            wt = lts[:, bt, :, i * 128 : (i + 1) * 128]
            for h in range(DW // MC):
                so = q * DW + h * MC
                ch, off = so // MCH, so % MCH
                nc.tensor.matmul(
                    pt[:, h * MC : (h + 1) * MC],
                    wt,
                    rtc[:, bt, ch, :, off : off + MC],
                    perf_mode=DR,
                )

        # Batches 0-1: chunk-major across the 4 row blocks, so chunk q+1
        # is first needed ~8 matmul-pairs after chunk q (slack for the
        # slow early DMA and the trickling deferred loads); 1024-wide
        # stores issued per drain.
        for bt in range(2):
            sts = [
                stage.tile([128, M], F16, tag="st", name=f"st{bt}_{i}")
                for i in range(NB)
            ]
            for q in range(M // DW):
                for i in range(NB):
                    pt = mpsum.tile(
                        [128, DW], F32, tag="mp", name=f"mp{bt}_{q}_{i}"
                    )
                    mm_pair(pt, bt, i, q)
                    drain(sts[i][:, q * DW : (q + 1) * DW], pt[:, :])
                    nc.sync.dma_start(
                        out=d_d[
                            bt, i * 128 : (i + 1) * 128, q * DW : (q + 1) * DW
                        ],
                        in_=sts[i][:, q * DW : (q + 1) * DW],
                    )

        # Batches 2-3: block-major, fat 4096-wide stores (except the
        # last block: 1024-wide pieces for a short final receipt tail).
        for bt in range(2, B):
            for i in range(NB):
                blk = bt * NB + i
                st = stage.tile([128, M], F16, tag="st", name=f"st{bt}_{i}")
                stw = 1024 if blk == B * NB - 1 else M
                for q in range(M // DW):
                    pt = mpsum.tile(
                        [128, DW], F32, tag="mp", name=f"mp{bt}_{i}_{q}"
                    )
                    mm_pair(pt, bt, i, q)
                    drain(st[:, q * DW : (q + 1) * DW], pt[:, :])
                    if ((q + 1) * DW) % stw == 0:
                        p = (q * DW) // stw
                        nc.sync.dma_start(
                            out=d_d[
                                bt,
                                i * 128 : (i + 1) * 128,
                                p * stw : (p + 1) * stw,
                            ],
                            in_=st[:, p * stw : (p + 1) * stw],
                        )

    nc.compile()
    return nc


def _get_nc():
    if "nc" not in _CACHE:
        _CACHE["nc"] = _build_nc()
    return _CACHE["nc"]


_F8NP = mybir.dt.np(F8)


def _q8(x):
    return np.clip(x, -240.0, 240.0).astype(_F8NP).astype(np.float32)


def _make_in_maps(a, b):
    a = np.asarray(a, dtype=np.float32)
    b = np.asarray(b, dtype=np.float32)
    aa = np.einsum("bcn,bcn->bn", a, a)  # [B, N]
    bb = np.einsum("bcm,bcm->bm", b, b)  # [B, M]

    c = -2.0 * a
    c_hi = _q8(c)
    c_lo = _q8(c - c_hi)
    b_hi = _q8(b)
    b_lo = _q8(b - b_hi)
    A = aa - 64.0
    A_hi = _q8(A)
    A_lo = _q8(A - A_hi)
    Bv = bb - 64.0
    B_hi = _q8(Bv)
    B_lo = _q8(Bv - B_hi)

    lhs = np.zeros([B, KAUG, N], dtype=np.float32)
    rhs = np.zeros([B, KAUG, M], dtype=np.float32)
    lhs[:, 0:64] = c_hi
    rhs[:, 0:64] = b_hi
    lhs[:, 64:128] = c_hi
    rhs[:, 64:128] = b_lo
    lhs[:, 128:192] = c_lo
    rhs[:, 128:192] = b_hi
    lhs[:, 192] = A_hi
    rhs[:, 192] = 1.0
    lhs[:, 193] = A_lo
    rhs[:, 193] = 1.0
    lhs[:, 194] = 1.0
    rhs[:, 194] = B_hi
    lhs[:, 195] = 1.0
    rhs[:, 195] = B_lo
    lhs[:, 196] = 1.0
    rhs[:, 196] = 128.0

    lhs8 = lhs.astype(_F8NP)   # values already on the fp8 grid -> exact
    rhs8 = rhs.astype(_F8NP)
    # fold K rows [200] -> [100, 2] with k = j2*100 + ki (DoubleRow pairing)
    lhs8 = lhs8.reshape(B, 2, KI, N).transpose(2, 0, 1, 3)  # [KI, B, 2, N]
    rhs8 = np.ascontiguousarray(
        rhs8.reshape(B, 2, KI, NCH, MCH).transpose(2, 0, 3, 1, 4)
    )  # [KI, B, NCH, 2, MCH]

    in_maps = []
    for k in range(NCORES):
        lk = lhs8[:, :, :, k * NSH : (k + 1) * NSH]
        in_maps.append(
            {
                "lhs": np.ascontiguousarray(lk),
                "rhs": rhs8,
            }
        )
    return in_maps


def kernel(a, b, _trace=False, _trace_kwargs=None):
    nc = _get_nc()
    in_maps = _make_in_maps(a, b)
    res = run_bass_kernel_spmd(
        nc,
        in_maps,
        core_ids=list(range(NCORES)),
        trace=_trace,
        **(_trace_kwargs or {}),
    )
    out = np.concatenate(
        [res.results[k]["d"] for k in range(NCORES)], axis=1
    ).astype(np.float32)
    if _trace:
        _CACHE["last_results"] = res
    return out
